# revision 5
# baseline (speedup 1.0000x reference)
"""DecoderRNN (teacher-forced GRU decoder + vocab projection + log_softmax)
on 8 Trainium2 NeuronCores.

Sharding: batch-split GRU recurrence (4 examples per core), vocab-split
output projection (4000 vocab per core). Hidden states are AllGathered in
two time-chunks; log-softmax normalization uses one small AllGather of
per-row sum(exp(logit)) per row-quarter (logits are O(1) so no max shift
is needed for fp32 exp).

Self-contained: hardcodes all shapes; takes full inputs, returns full
outputs (log_probs [32,64,32000] f32, hidden [1,32,1024] f32).
"""
import numpy as np
import ml_dtypes

import concourse.bass as bass
import concourse.bacc as bacc
import concourse.tile as tile
from concourse import mybir
from concourse import bass_utils
from concourse.masks import make_identity

BF = mybir.dt.bfloat16
F32 = mybir.dt.float32
I32 = mybir.dt.int32

NCORES = 8
B, T, H, V = 32, 64, 1024, 32000
BPC = B // NCORES          # 4 examples per core
VPC = V // NCORES          # 4000 vocab per core
ROWS = B * T               # 2048
RPC = BPC * T              # 256 rows gathered per core
KC = H // 128              # 8 k-chunks
MC3 = 3 * H // 128         # 24 m-chunks of gates
NT = 16                    # projection row-tiles of 128
NV = (VPC + 511) // 512    # 8 vocab chunks (last = 416)
SOS = 1

_CACHE = {}


def _build():
    nc = bacc.Bacc("TRN2", target_bir_lowering=False, debug=False,
                   num_devices=NCORES)
    emb = nc.dram_tensor("emb", [V, H], F32, kind="ExternalInput").ap()
    idx = nc.dram_tensor("idx", [RPC], I32, kind="ExternalInput").ap()
    wih_t = nc.dram_tensor("wih_t", [H, 3 * H], BF, kind="ExternalInput").ap()
    whh_t = nc.dram_tensor("whh_t", [H, 3 * H], BF, kind="ExternalInput").ap()
    b_ihT = nc.dram_tensor("b_ihT", [128, MC3], F32, kind="ExternalInput").ap()
    b_hhB = nc.dram_tensor("b_hhB", [128, MC3, BPC], F32, kind="ExternalInput").ap()
    h0_t = nc.dram_tensor("h0_t", [H, BPC], F32, kind="ExternalInput").ap()
    oww = nc.dram_tensor("oww", [H, VPC], BF, kind="ExternalInput").ap()
    ob = nc.dram_tensor("ob", [VPC], F32, kind="ExternalInput").ap()
    out_lp = nc.dram_tensor("out_lp", [ROWS, VPC], F32, kind="ExternalOutput").ap()
    out_h = nc.dram_tensor("out_h", [H, BPC], F32, kind="ExternalOutput").ap()

    with tile.TileContext(nc) as tc:
        with tc.tile_pool(name="pg", bufs=1) as pg, \
             tc.tile_pool(name="pg_dram", bufs=1, space="DRAM") as pgd:
            # ---- long-lived tiles ----
            hsall = pg.tile([128, KC, NCORES, T, BPC], BF)   # gathered hidden states
            slab = pg.tile([128, KC, T, BPC], BF)            # own hidden states
            hT = pg.tile([128, KC, BPC], F32)                # current h (fp32)

            nc.sync.dma_start(hT[:], h0_t.rearrange("(a p) b -> p a b", p=128))

            # DRAM bounce buffers for collectives
            agin1 = pgd.tile([128, KC, 32, BPC], BF)
            agin2 = pgd.tile([128, KC, 32, BPC], BF)
            agout1 = pgd.tile([NCORES, 128, KC, 32, BPC], BF, addr_space="Shared")
            agout2 = pgd.tile([NCORES, 128, KC, 32, BPC], BF, addr_space="Shared")
            ags_in = [pgd.tile([128, 4], F32, name=f"ags_in{q}")
                      for q in range(4)]
            ags_out = [pgd.tile([NCORES, 128, 4], F32, addr_space="Shared",
                                name=f"ags_out{q}") for q in range(4)]

            pab_ctx = tc.tile_pool(name="pab", bufs=1)
            pab = pab_ctx.__enter__()
            hbf0 = pab.tile([128, KC, BPC], BF)              # h0 in bf16
            bihT = pab.tile([128, MC3], F32)
            bhhB = pab.tile([128, MC3, BPC], F32)
            idx_sb = pab.tile([128, 2], I32)
            xg_sb = pab.tile([128, MC3, RPC], F32)           # xg' = x@W_ih.T + b_ih
            whh_sb = pab.tile([128, KC, 3 * H], BF)
            nc.sync.dma_start(whh_sb[:], whh_t.rearrange("(a p) m -> p a m", p=128))

            nc.sync.dma_start(bihT[:], b_ihT[:])
            nc.sync.dma_start(bhhB[:], b_hhB[:])
            nc.sync.dma_start(idx_sb[:], idx.rearrange("(j p) -> p j", p=128))
            nc.vector.tensor_copy(hbf0[:], hT[:])

            # ================= Phase A: gather + relu + transpose + xg =====
            with nc.named_scope("ph_a"), \
                 tc.tile_pool(name="pa", bufs=1) as pa, \
                 tc.tile_pool(name="pa_ps", bufs=2, space="PSUM") as paps, \
                 tc.tile_pool(name="pa_ps2", bufs=2, space="PSUM") as paps2:
                x_sb = pa.tile([128, 2, H], F32)
                ident = pa.tile([128, 128], F32)
                make_identity(nc, ident[:])
                for j in range(2):
                    nc.gpsimd.indirect_dma_start(
                        out=x_sb[:, j, :], out_offset=None,
                        in_=emb[:],
                        in_offset=bass.IndirectOffsetOnAxis(ap=idx_sb[:, j:j + 1], axis=0),
                    )
                wih_sb = pa.tile([128, KC, 3 * H], BF)
                nc.sync.dma_start(wih_sb[:], wih_t.rearrange("(a p) m -> p a m", p=128))

                # xT[p, kc, r] = relu(x[r, 128*kc+p]) for r = 4t + bl
                xT = pa.tile([128, KC, RPC], BF)
                for j in range(2):
                    for k in range(KC):
                        tp = paps.tile([128, 128], F32, tag="tp")
                        nc.tensor.transpose(tp[:], x_sb[:, j, 128 * k:128 * k + 128],
                                            ident[:])
                        nc.scalar.activation(xT[:, k, 128 * j:128 * j + 128], tp[:],
                                             mybir.ActivationFunctionType.Relu)

                # xg'[p, m, r] = sum_k W_ih[128m+p, k] x[r, k] + b_ih[128m+p]
                for m in range(MC3):
                    pxg = paps2.tile([128, RPC], F32, tag="pxg")
                    for k in range(KC):
                        nc.tensor.matmul(pxg[:], wih_sb[:, k, 128 * m:128 * m + 128],
                                         xT[:, k, :],
                                         start=(k == 0), stop=(k == KC - 1))
                    nc.vector.tensor_add(
                        out=xg_sb[:, m, :], in0=pxg[:],
                        in1=bihT[:, m:m + 1].to_broadcast([128, RPC]))

            # ================= Phase B: GRU recurrence =====================
            with nc.named_scope("rec"), \
                 tc.tile_pool(name="pb", bufs=3) as pb, \
                 tc.tile_pool(name="pb_ps", bufs=2, space="PSUM") as pbps:
                for t in range(T):
                    rhs = hbf0 if t == 0 else slab[:, :, t - 1, :]
                    hg = pbps.tile([128, MC3, BPC], F32, tag="hg")
                    for m in range(MC3):
                        for k in range(KC):
                            nc.tensor.matmul(
                                hg[:, m, :], whh_sb[:, k, 128 * m:128 * m + 128],
                                (rhs[:, k, :] if t == 0 else slab[:, k, t - 1, :]),
                                start=(k == 0), stop=(k == KC - 1))
                    # gates (fp32)
                    hgb = pb.tile([128, MC3, BPC], F32, tag="hgb")
                    nc.vector.tensor_add(out=hgb[:], in0=hg[:], in1=bhhB[:])
                    rzs = pb.tile([128, 16, BPC], F32, tag="rzs")
                    nc.vector.tensor_add(out=rzs[:], in0=hgb[:, 0:16, :],
                                         in1=xg_sb[:, 0:16, 4 * t:4 * t + BPC])
                    rz = pb.tile([128, 16, BPC], F32, tag="rz")
                    nc.scalar.activation(rz[:], rzs[:],
                                         mybir.ActivationFunctionType.Sigmoid)
                    ns = pb.tile([128, KC, BPC], F32, tag="ns")
                    nc.vector.tensor_mul(out=ns[:], in0=rz[:, 0:KC, :],
                                         in1=hgb[:, 16:24, :])
                    nc.vector.tensor_add(out=ns[:], in0=ns[:],
                                         in1=xg_sb[:, 16:24, 4 * t:4 * t + BPC])
                    n_t = pb.tile([128, KC, BPC], F32, tag="n_t")
                    nc.scalar.activation(n_t[:], ns[:],
                                         mybir.ActivationFunctionType.Tanh)
                    d = pb.tile([128, KC, BPC], F32, tag="d")
                    nc.vector.tensor_tensor(out=d[:], in0=hT[:], in1=n_t[:],
                                            op=mybir.AluOpType.subtract)
                    nc.vector.tensor_mul(out=d[:], in0=d[:], in1=rz[:, 8:16, :])
                    nc.vector.tensor_add(out=hT[:], in0=n_t[:], in1=d[:])
                    nc.vector.tensor_copy(slab[:, :, t, :], hT[:])

                    if t == 31:
                        nc.sync.dma_start(agin1[:], slab[:, :, 0:32, :])
                        nc.gpsimd.collective_compute(
                            "AllGather", mybir.AluOpType.bypass,
                            replica_groups=[list(range(NCORES))],
                            ins=[agin1.opt()], outs=[agout1.opt()])
                        for c in range(NCORES):
                            nc.sync.dma_start(hsall[:, :, c, 0:32, :], agout1[c])
                nc.sync.dma_start(out_h.rearrange("(a p) b -> p a b", p=128), hT[:])
                nc.sync.dma_start(agin2[:], slab[:, :, 32:64, :])
                nc.gpsimd.collective_compute(
                    "AllGather", mybir.AluOpType.bypass,
                    replica_groups=[list(range(NCORES))],
                    ins=[agin2.opt()], outs=[agout2.opt()])
                for c in range(NCORES):
                    nc.sync.dma_start(hsall[:, :, c, 32:64, :], agout2[c])

            pab_ctx.__exit__(None, None, None)

            # ================= Phase C: projection + log_softmax ===========
            with nc.named_scope("proj"), \
                 tc.tile_pool(name="pc", bufs=1) as pc, \
                 tc.tile_pool(name="pc_sl", bufs=6) as pcs, \
                 tc.tile_pool(name="pc_sm", bufs=4) as pcm, \
                 tc.tile_pool(name="pc_ps", bufs=4, space="PSUM") as pcps:
                oww_sb = pc.tile([128, KC, VPC], BF)
                nc.sync.dma_start(oww_sb[:], oww.rearrange("(a p) v -> p a v", p=128))
                obB = pc.tile([128, VPC], F32)
                nc.sync.dma_start(obB[:], ob[None, :].to_broadcast([128, VPC]))

                q_tiles = []  # slabs of current quarter
                for mt in range(NT):
                    c2, half = mt // 2, mt % 2
                    mq = mt % 4
                    if mq == 0:
                        s_q = pcm.tile([128, 4], F32, tag="s_q")
                        q_tiles = []
                    lg = pcs.tile([128, VPC], BF, tag="lg")
                    q_tiles.append(lg)
                    spart = pcm.tile([128, NV], F32, tag="spart")
                    for n in range(NV):
                        nn = min(512, VPC - 512 * n)
                        pp = pcps.tile([128, 512], F32, tag="pp")
                        for k in range(KC):
                            nc.tensor.matmul(
                                pp[:, :nn],
                                hsall[:, k, c2, 32 * half:32 * half + 32, :],
                                oww_sb[:, k, 512 * n:512 * n + nn],
                                start=(k == 0), stop=(k == KC - 1))
                        nc.vector.tensor_add(out=lg[:, 512 * n:512 * n + nn],
                                             in0=pp[:, :nn],
                                             in1=obB[:, 512 * n:512 * n + nn])
                        esc = pcm.tile([128, 512], F32, tag="esc")
                        nc.scalar.activation(esc[:, :nn], lg[:, 512 * n:512 * n + nn],
                                             mybir.ActivationFunctionType.Exp,
                                             accum_out=spart[:, n:n + 1])
                    nc.vector.reduce_sum(s_q[:, mq:mq + 1], spart[:],
                                         axis=mybir.AxisListType.X)

                    if mq == 3:
                        qq = mt // 4
                        nc.sync.dma_start(ags_in[qq][:], s_q[:])
                        nc.gpsimd.collective_compute(
                            "AllGather", mybir.AluOpType.bypass,
                            replica_groups=[list(range(NCORES))],
                            ins=[ags_in[qq].opt()], outs=[ags_out[qq].opt()])
                        sall = pcm.tile([128, 4, NCORES], F32, tag="sall")
                        nc.sync.dma_start(
                            sall[:], ags_out[qq][:].rearrange("c p m -> p m c"))
                        s_g = pcm.tile([128, 4], F32, tag="s_g")
                        nc.vector.reduce_sum(s_g[:], sall[:],
                                             axis=mybir.AxisListType.X)
                        nb = pcm.tile([128, 4], F32, tag="nb")
                        nc.scalar.activation(nb[:], s_g[:],
                                             mybir.ActivationFunctionType.Ln)
                        nc.vector.tensor_scalar_mul(nb[:], nb[:], -1.0)
                        # normalize + write out the 4 row-tiles of this quarter
                        for i, lgq in enumerate(q_tiles):
                            mtq = 4 * qq + i
                            for n in range(NV):
                                nn = min(512, VPC - 512 * n)
                                ofl = pcm.tile([128, 512], F32, tag="ofl")
                                nc.scalar.activation(
                                    ofl[:, :nn], lgq[:, 512 * n:512 * n + nn],
                                    mybir.ActivationFunctionType.Identity,
                                    bias=nb[:, i:i + 1])
                                nc.sync.dma_start(
                                    out_lp[128 * mtq:128 * mtq + 128,
                                           512 * n:512 * n + nn],
                                    ofl[:, :nn])
    nc.compile()
    return nc


def _get_nc():
    if "nc" not in _CACHE:
        _CACHE["nc"] = _build()
    return _CACHE["nc"]


def kernel(encoder_outputs, encoder_hidden, target_tensor, embedding,
           W_ih, W_hh, b_ih, b_hh, out_W, out_b):
    del encoder_outputs
    bf = ml_dtypes.bfloat16
    emb = np.ascontiguousarray(np.asarray(embedding, dtype=np.float32))
    tt = np.asarray(target_tensor)
    eh = np.asarray(encoder_hidden, dtype=np.float32)
    wih_t = np.ascontiguousarray(np.asarray(W_ih, np.float32).T).astype(bf)
    whh_t = np.ascontiguousarray(np.asarray(W_hh, np.float32).T).astype(bf)
    b_ih = np.asarray(b_ih, np.float32)
    b_hh = np.asarray(b_hh, np.float32)
    out_W = np.asarray(out_W, np.float32)
    out_b = np.asarray(out_b, np.float32)

    b_ihT = np.ascontiguousarray(b_ih.reshape(MC3, 128).T)
    b_hhB = np.ascontiguousarray(
        np.broadcast_to(b_hh.reshape(MC3, 128).T[:, :, None], (128, MC3, BPC)))

    # teacher-forcing inputs: SOS then target[:, :-1]
    ins = np.concatenate(
        [np.full((B, 1), SOS, dtype=np.int64), tt[:, :-1].astype(np.int64)], axis=1)

    in_maps = []
    for c in range(NCORES):
        sl = slice(BPC * c, BPC * (c + 1))
        vsl = slice(VPC * c, VPC * (c + 1))
        idx_c = np.ascontiguousarray(ins[sl].T.reshape(-1)).astype(np.int32)
        h0_t = np.ascontiguousarray(eh[0, sl].T)
        oww = np.ascontiguousarray(out_W[vsl].T).astype(bf)
        in_maps.append({
            "emb": emb, "idx": idx_c, "wih_t": wih_t, "whh_t": whh_t,
            "b_ihT": b_ihT, "b_hhB": b_hhB, "h0_t": h0_t,
            "oww": oww, "ob": np.ascontiguousarray(out_b[vsl]),
        })

    nc = _get_nc()
    res = bass_utils.run_bass_kernel_spmd(
        nc, in_maps, core_ids=list(range(NCORES)),
        trace=bool(_CACHE.get("trace")))
    _CACHE["last_results"] = res

    lp_parts = []
    for c in range(NCORES):
        r = res.results[c]["out_lp"]
        # device rows: [c2, half, t32, b, V] -> [b_global, t, V]
        r = r.reshape(NCORES, 2, 32, BPC, VPC).transpose(0, 3, 1, 2, 4)
        lp_parts.append(r.reshape(B, T, VPC))
    log_probs = np.concatenate(lp_parts, axis=-1)
    hidden = np.concatenate(
        [res.results[c]["out_h"].T for c in range(NCORES)], axis=0)[None]
    return log_probs, hidden


# revision 6
# speedup vs baseline: 1.0082x; 1.0082x over previous
"""DecoderRNN (teacher-forced GRU decoder + vocab projection + log_softmax)
on 8 Trainium2 NeuronCores.

Sharding: batch-split GRU recurrence (4 examples per core), vocab-split
output projection (4000 vocab per core). Hidden states are AllGathered in
two time-chunks; log-softmax normalization uses one small AllGather of
per-row sum(exp(logit)) per row-quarter (logits are O(1) so no max shift
is needed for fp32 exp).

Self-contained: hardcodes all shapes; takes full inputs, returns full
outputs (log_probs [32,64,32000] f32, hidden [1,32,1024] f32).
"""
import numpy as np
import ml_dtypes

import concourse.bass as bass
import concourse.bacc as bacc
import concourse.tile as tile
from concourse import mybir
from concourse import bass_utils
from concourse.masks import make_identity

BF = mybir.dt.bfloat16
F32 = mybir.dt.float32
I32 = mybir.dt.int32

NCORES = 8
B, T, H, V = 32, 64, 1024, 32000
BPC = B // NCORES          # 4 examples per core
VPC = V // NCORES          # 4000 vocab per core
ROWS = B * T               # 2048
RPC = BPC * T              # 256 rows gathered per core
KC = H // 128              # 8 k-chunks
MC3 = 3 * H // 128         # 24 m-chunks of gates
NT = 16                    # projection row-tiles of 128
NV = (VPC + 511) // 512    # 8 vocab chunks (last = 416)
SOS = 1

_CACHE = {}


def _build():
    nc = bacc.Bacc("TRN2", target_bir_lowering=False, debug=False,
                   num_devices=NCORES)
    emb = nc.dram_tensor("emb", [V, H], F32, kind="ExternalInput").ap()
    idx = nc.dram_tensor("idx", [RPC], I32, kind="ExternalInput").ap()
    wih_t = nc.dram_tensor("wih_t", [H, 3 * H], BF, kind="ExternalInput").ap()
    whh_t = nc.dram_tensor("whh_t", [H, 3 * H], BF, kind="ExternalInput").ap()
    b_ihT = nc.dram_tensor("b_ihT", [128, MC3], F32, kind="ExternalInput").ap()
    b_hhB = nc.dram_tensor("b_hhB", [128, MC3, BPC], F32, kind="ExternalInput").ap()
    h0_t = nc.dram_tensor("h0_t", [H, BPC], F32, kind="ExternalInput").ap()
    oww = nc.dram_tensor("oww", [H, VPC], BF, kind="ExternalInput").ap()
    ob = nc.dram_tensor("ob", [VPC], F32, kind="ExternalInput").ap()
    out_lp = nc.dram_tensor("out_lp", [ROWS, VPC], F32, kind="ExternalOutput").ap()
    out_h = nc.dram_tensor("out_h", [H, BPC], F32, kind="ExternalOutput").ap()

    with tile.TileContext(nc) as tc:
        with tc.tile_pool(name="pg", bufs=1) as pg, \
             tc.tile_pool(name="pg_dram", bufs=1, space="DRAM") as pgd:
            # ---- long-lived tiles ----
            hsall = pg.tile([128, KC, NCORES, T, BPC], BF)   # gathered hidden states
            slab = pg.tile([128, KC, T, BPC], BF)            # own hidden states
            hT = pg.tile([128, KC, BPC], F32)                # current h (fp32)

            nc.sync.dma_start(hT[:], h0_t.rearrange("(a p) b -> p a b", p=128))

            # DRAM bounce buffers for collectives
            agin1 = pgd.tile([128, KC, 32, BPC], BF)
            agin2 = pgd.tile([128, KC, 32, BPC], BF)
            agout1 = pgd.tile([NCORES, 128, KC, 32, BPC], BF, addr_space="Shared")
            agout2 = pgd.tile([NCORES, 128, KC, 32, BPC], BF, addr_space="Shared")
            ags_in = [pgd.tile([128, 4], F32, name=f"ags_in{q}")
                      for q in range(4)]
            ags_out = [pgd.tile([NCORES, 128, 4], F32, addr_space="Shared",
                                name=f"ags_out{q}") for q in range(4)]

            pab_ctx = tc.tile_pool(name="pab", bufs=1)
            pab = pab_ctx.__enter__()
            hbf0 = pab.tile([128, KC, BPC], BF)              # h0 in bf16
            bihT = pab.tile([128, MC3], F32)
            bhhB = pab.tile([128, MC3, BPC], F32)
            idx_sb = pab.tile([128, 2], I32)
            xg_sb = pab.tile([128, MC3, RPC], F32)           # xg' = x@W_ih.T + b_ih
            whh_sb = pab.tile([128, KC, 3 * H], BF)
            nc.sync.dma_start(whh_sb[:], whh_t.rearrange("(a p) m -> p a m", p=128))

            nc.sync.dma_start(bihT[:], b_ihT[:])
            nc.sync.dma_start(bhhB[:], b_hhB[:])
            nc.sync.dma_start(idx_sb[:], idx.rearrange("(j p) -> p j", p=128))
            nc.vector.tensor_copy(hbf0[:], hT[:])

            # ================= Phase A: gather + relu + transpose + xg =====
            with nc.named_scope("ph_a"), \
                 tc.tile_pool(name="pa", bufs=1) as pa, \
                 tc.tile_pool(name="pa_ps", bufs=2, space="PSUM") as paps, \
                 tc.tile_pool(name="pa_ps2", bufs=2, space="PSUM") as paps2:
                x_sb = pa.tile([128, 2, H], F32)
                ident = pa.tile([128, 128], F32)
                make_identity(nc, ident[:])
                for j in range(2):
                    nc.gpsimd.indirect_dma_start(
                        out=x_sb[:, j, :], out_offset=None,
                        in_=emb[:],
                        in_offset=bass.IndirectOffsetOnAxis(ap=idx_sb[:, j:j + 1], axis=0),
                    )
                wih_sb = pa.tile([128, KC, 3 * H], BF)
                nc.sync.dma_start(wih_sb[:], wih_t.rearrange("(a p) m -> p a m", p=128))

                # xT[p, kc, r] = relu(x[r, 128*kc+p]) for r = 4t + bl
                xT = pa.tile([128, KC, RPC], BF)
                for j in range(2):
                    for k in range(KC):
                        tp = paps.tile([128, 128], F32, tag="tp")
                        nc.tensor.transpose(tp[:], x_sb[:, j, 128 * k:128 * k + 128],
                                            ident[:])
                        nc.scalar.activation(xT[:, k, 128 * j:128 * j + 128], tp[:],
                                             mybir.ActivationFunctionType.Relu)

                # xg'[p, m, r] = sum_k W_ih[128m+p, k] x[r, k] + b_ih[128m+p]
                for m in range(MC3):
                    pxg = paps2.tile([128, RPC], F32, tag="pxg")
                    for k in range(KC):
                        nc.tensor.matmul(pxg[:], wih_sb[:, k, 128 * m:128 * m + 128],
                                         xT[:, k, :],
                                         start=(k == 0), stop=(k == KC - 1))
                    nc.vector.tensor_add(
                        out=xg_sb[:, m, :], in0=pxg[:],
                        in1=bihT[:, m:m + 1].to_broadcast([128, RPC]))

            # ================= Phase B: GRU recurrence =====================
            with nc.named_scope("rec"), \
                 tc.tile_pool(name="pb", bufs=3) as pb, \
                 tc.tile_pool(name="pb_ps", bufs=2, space="PSUM") as pbps:
                for t in range(T):
                    rhs = hbf0 if t == 0 else slab[:, :, t - 1, :]
                    hg = pbps.tile([128, MC3, BPC], F32, tag="hg")
                    for m in range(MC3):
                        for k in range(KC):
                            nc.tensor.matmul(
                                hg[:, m, :], whh_sb[:, k, 128 * m:128 * m + 128],
                                (rhs[:, k, :] if t == 0 else slab[:, k, t - 1, :]),
                                start=(k == 0), stop=(k == KC - 1))
                    # gates (fp32)
                    hgb = pb.tile([128, MC3, BPC], F32, tag="hgb")
                    nc.vector.tensor_add(out=hgb[:], in0=hg[:], in1=bhhB[:])
                    rzs = pb.tile([128, 16, BPC], F32, tag="rzs")
                    nc.vector.tensor_add(out=rzs[:], in0=hgb[:, 0:16, :],
                                         in1=xg_sb[:, 0:16, 4 * t:4 * t + BPC])
                    rz = pb.tile([128, 16, BPC], F32, tag="rz")
                    nc.scalar.activation(rz[:], rzs[:],
                                         mybir.ActivationFunctionType.Sigmoid)
                    ns = pb.tile([128, KC, BPC], F32, tag="ns")
                    nc.vector.tensor_mul(out=ns[:], in0=rz[:, 0:KC, :],
                                         in1=hgb[:, 16:24, :])
                    nc.vector.tensor_add(out=ns[:], in0=ns[:],
                                         in1=xg_sb[:, 16:24, 4 * t:4 * t + BPC])
                    n_t = pb.tile([128, KC, BPC], F32, tag="n_t")
                    nc.scalar.activation(n_t[:], ns[:],
                                         mybir.ActivationFunctionType.Tanh)
                    d = pb.tile([128, KC, BPC], F32, tag="d")
                    nc.vector.tensor_tensor(out=d[:], in0=hT[:], in1=n_t[:],
                                            op=mybir.AluOpType.subtract)
                    nc.vector.tensor_mul(out=d[:], in0=d[:], in1=rz[:, 8:16, :])
                    nc.vector.tensor_add(out=hT[:], in0=n_t[:], in1=d[:])
                    nc.vector.tensor_copy(slab[:, :, t, :], hT[:])

                    if t == 31:
                        nc.sync.dma_start(agin1[:], slab[:, :, 0:32, :])
                        nc.gpsimd.collective_compute(
                            "AllGather", mybir.AluOpType.bypass,
                            replica_groups=[list(range(NCORES))],
                            ins=[agin1.opt()], outs=[agout1.opt()])
                        for c in range(NCORES):
                            nc.sync.dma_start(hsall[:, :, c, 0:32, :], agout1[c])
                nc.sync.dma_start(out_h.rearrange("(a p) b -> p a b", p=128), hT[:])
                nc.sync.dma_start(agin2[:], slab[:, :, 32:64, :])
                nc.gpsimd.collective_compute(
                    "AllGather", mybir.AluOpType.bypass,
                    replica_groups=[list(range(NCORES))],
                    ins=[agin2.opt()], outs=[agout2.opt()])
                for c in range(NCORES):
                    nc.sync.dma_start(hsall[:, :, c, 32:64, :], agout2[c])

            pab_ctx.__exit__(None, None, None)

            # ================= Phase C: projection + log_softmax ===========
            with nc.named_scope("proj"), \
                 tc.tile_pool(name="pc", bufs=1) as pc, \
                 tc.tile_pool(name="pc_sl", bufs=8) as pcs, \
                 tc.tile_pool(name="pc_sm", bufs=4) as pcm, \
                 tc.tile_pool(name="pc_ps", bufs=2, space="PSUM") as pcps:
                oww_sb = pc.tile([128, KC, VPC], BF)
                nc.sync.dma_start(oww_sb[:], oww.rearrange("(a p) v -> p a v", p=128))
                obB = pc.tile([128, VPC], F32)
                nc.sync.dma_start(obB[:], ob[None, :].to_broadcast([128, VPC]))

                q_tiles = []  # slabs of current quarter
                for mt in range(NT):
                    c2, half = mt // 2, mt % 2
                    mq = mt % 4
                    if mq == 0:
                        s_q = pcm.tile([128, 4], F32, tag="s_q")
                        q_tiles = []
                    lg = pcs.tile([128, VPC], BF, tag="lg")
                    q_tiles.append(lg)
                    spart = pcm.tile([128, NV], F32, tag="spart")
                    for g in range(NV // 4):
                        pp = pcps.tile([128, 4, 512], F32, tag="pp")
                        for k in range(KC):
                            for j in range(4):
                                n = 4 * g + j
                                nn = min(512, VPC - 512 * n)
                                nc.tensor.matmul(
                                    pp[:, j, :nn],
                                    hsall[:, k, c2, 32 * half:32 * half + 32, :],
                                    oww_sb[:, k, 512 * n:512 * n + nn],
                                    start=(k == 0), stop=(k == KC - 1))
                        for j in range(4):
                            n = 4 * g + j
                            nn = min(512, VPC - 512 * n)
                            nc.vector.tensor_add(out=lg[:, 512 * n:512 * n + nn],
                                                 in0=pp[:, j, :nn],
                                                 in1=obB[:, 512 * n:512 * n + nn])
                            esc = pcm.tile([128, 512], F32, tag="esc")
                            nc.scalar.activation(esc[:, :nn],
                                                 lg[:, 512 * n:512 * n + nn],
                                                 mybir.ActivationFunctionType.Exp,
                                                 accum_out=spart[:, n:n + 1])
                    nc.vector.reduce_sum(s_q[:, mq:mq + 1], spart[:],
                                         axis=mybir.AxisListType.X)

                    if mq == 3:
                        qq = mt // 4
                        nc.sync.dma_start(ags_in[qq][:], s_q[:])
                        nc.gpsimd.collective_compute(
                            "AllGather", mybir.AluOpType.bypass,
                            replica_groups=[list(range(NCORES))],
                            ins=[ags_in[qq].opt()], outs=[ags_out[qq].opt()])
                        sall = pcm.tile([128, 4, NCORES], F32, tag="sall")
                        nc.sync.dma_start(
                            sall[:], ags_out[qq][:].rearrange("c p m -> p m c"))
                        s_g = pcm.tile([128, 4], F32, tag="s_g")
                        nc.vector.reduce_sum(s_g[:], sall[:],
                                             axis=mybir.AxisListType.X)
                        nb = pcm.tile([128, 4], F32, tag="nb")
                        nc.scalar.activation(nb[:], s_g[:],
                                             mybir.ActivationFunctionType.Ln)
                        nc.vector.tensor_scalar_mul(nb[:], nb[:], -1.0)
                        # normalize + write out the 4 row-tiles of this quarter
                        for i, lgq in enumerate(q_tiles):
                            mtq = 4 * qq + i
                            for n in range(NV):
                                nn = min(512, VPC - 512 * n)
                                ofl = pcm.tile([128, 512], F32, tag="ofl")
                                nc.scalar.activation(
                                    ofl[:, :nn], lgq[:, 512 * n:512 * n + nn],
                                    mybir.ActivationFunctionType.Identity,
                                    bias=nb[:, i:i + 1])
                                nc.sync.dma_start(
                                    out_lp[128 * mtq:128 * mtq + 128,
                                           512 * n:512 * n + nn],
                                    ofl[:, :nn])
    nc.compile()
    return nc


def _get_nc():
    if "nc" not in _CACHE:
        _CACHE["nc"] = _build()
    return _CACHE["nc"]


def kernel(encoder_outputs, encoder_hidden, target_tensor, embedding,
           W_ih, W_hh, b_ih, b_hh, out_W, out_b):
    del encoder_outputs
    bf = ml_dtypes.bfloat16
    emb = np.ascontiguousarray(np.asarray(embedding, dtype=np.float32))
    tt = np.asarray(target_tensor)
    eh = np.asarray(encoder_hidden, dtype=np.float32)
    wih_t = np.ascontiguousarray(np.asarray(W_ih, np.float32).T).astype(bf)
    whh_t = np.ascontiguousarray(np.asarray(W_hh, np.float32).T).astype(bf)
    b_ih = np.asarray(b_ih, np.float32)
    b_hh = np.asarray(b_hh, np.float32)
    out_W = np.asarray(out_W, np.float32)
    out_b = np.asarray(out_b, np.float32)

    b_ihT = np.ascontiguousarray(b_ih.reshape(MC3, 128).T)
    b_hhB = np.ascontiguousarray(
        np.broadcast_to(b_hh.reshape(MC3, 128).T[:, :, None], (128, MC3, BPC)))

    # teacher-forcing inputs: SOS then target[:, :-1]
    ins = np.concatenate(
        [np.full((B, 1), SOS, dtype=np.int64), tt[:, :-1].astype(np.int64)], axis=1)

    in_maps = []
    for c in range(NCORES):
        sl = slice(BPC * c, BPC * (c + 1))
        vsl = slice(VPC * c, VPC * (c + 1))
        idx_c = np.ascontiguousarray(ins[sl].T.reshape(-1)).astype(np.int32)
        h0_t = np.ascontiguousarray(eh[0, sl].T)
        oww = np.ascontiguousarray(out_W[vsl].T).astype(bf)
        in_maps.append({
            "emb": emb, "idx": idx_c, "wih_t": wih_t, "whh_t": whh_t,
            "b_ihT": b_ihT, "b_hhB": b_hhB, "h0_t": h0_t,
            "oww": oww, "ob": np.ascontiguousarray(out_b[vsl]),
        })

    nc = _get_nc()
    res = bass_utils.run_bass_kernel_spmd(
        nc, in_maps, core_ids=list(range(NCORES)),
        trace=bool(_CACHE.get("trace")))
    _CACHE["last_results"] = res

    lp_parts = []
    for c in range(NCORES):
        r = res.results[c]["out_lp"]
        # device rows: [c2, half, t32, b, V] -> [b_global, t, V]
        r = r.reshape(NCORES, 2, 32, BPC, VPC).transpose(0, 3, 1, 2, 4)
        lp_parts.append(r.reshape(B, T, VPC))
    log_probs = np.concatenate(lp_parts, axis=-1)
    hidden = np.concatenate(
        [res.results[c]["out_h"].T for c in range(NCORES)], axis=0)[None]
    return log_probs, hidden


# revision 8
# speedup vs baseline: 1.0106x; 1.0024x over previous
"""DecoderRNN (teacher-forced GRU decoder + vocab projection + log_softmax)
on 8 Trainium2 NeuronCores.

Sharding: batch-split GRU recurrence (4 examples per core), vocab-split
output projection (4000 vocab per core). Hidden states are AllGathered in
two time-chunks; log-softmax normalization uses one small AllGather of
per-row sum(exp(logit)) per row-quarter (logits are O(1) so no max shift
is needed for fp32 exp).

Self-contained: hardcodes all shapes; takes full inputs, returns full
outputs (log_probs [32,64,32000] f32, hidden [1,32,1024] f32).
"""
import numpy as np
import ml_dtypes

import concourse.bass as bass
import concourse.bacc as bacc
import concourse.tile as tile
from concourse import mybir
from concourse import bass_utils
from concourse.masks import make_identity

BF = mybir.dt.bfloat16
F32 = mybir.dt.float32
I32 = mybir.dt.int32

NCORES = 8
B, T, H, V = 32, 64, 1024, 32000
BPC = B // NCORES          # 4 examples per core
VPC = V // NCORES          # 4000 vocab per core
ROWS = B * T               # 2048
RPC = BPC * T              # 256 rows gathered per core
KC = H // 128              # 8 k-chunks
MC3 = 3 * H // 128         # 24 m-chunks of gates
NT = 16                    # projection row-tiles of 128
NV = (VPC + 511) // 512    # 8 vocab chunks (last = 416)
SOS = 1

_CACHE = {}


def _build():
    nc = bacc.Bacc("TRN2", target_bir_lowering=False, debug=False,
                   num_devices=NCORES)
    emb = nc.dram_tensor("emb", [V, H], F32, kind="ExternalInput").ap()
    idx = nc.dram_tensor("idx", [RPC], I32, kind="ExternalInput").ap()
    wih_t = nc.dram_tensor("wih_t", [H, 3 * H], BF, kind="ExternalInput").ap()
    whh_t = nc.dram_tensor("whh_t", [H, 3 * H], BF, kind="ExternalInput").ap()
    b_ihT = nc.dram_tensor("b_ihT", [128, MC3], F32, kind="ExternalInput").ap()
    b_hhB = nc.dram_tensor("b_hhB", [128, MC3, BPC], F32, kind="ExternalInput").ap()
    h0_t = nc.dram_tensor("h0_t", [H, BPC], F32, kind="ExternalInput").ap()
    oww = nc.dram_tensor("oww", [H, VPC], BF, kind="ExternalInput").ap()
    ob = nc.dram_tensor("ob", [VPC], F32, kind="ExternalInput").ap()
    out_lp = nc.dram_tensor("out_lp", [ROWS, VPC], F32, kind="ExternalOutput").ap()
    out_h = nc.dram_tensor("out_h", [H, BPC], F32, kind="ExternalOutput").ap()

    with tile.TileContext(nc) as tc:
        with tc.tile_pool(name="pg", bufs=1) as pg, \
             tc.tile_pool(name="pg_dram", bufs=1, space="DRAM") as pgd:
            # ---- long-lived tiles ----
            hsall = pg.tile([128, KC, NCORES, T, BPC], BF)   # gathered hidden states
            slab = pg.tile([128, KC, T, BPC], BF)            # own hidden states
            hT = pg.tile([128, KC, BPC], F32)                # current h (fp32)

            nc.sync.dma_start(hT[:], h0_t.rearrange("(a p) b -> p a b", p=128))

            # DRAM bounce buffers for collectives
            agin1 = pgd.tile([128, KC, 32, BPC], BF)
            agin2 = pgd.tile([128, KC, 32, BPC], BF)
            agout1 = pgd.tile([NCORES, 128, KC, 32, BPC], BF, addr_space="Shared")
            agout2 = pgd.tile([NCORES, 128, KC, 32, BPC], BF, addr_space="Shared")
            ags_in = [pgd.tile([128, 4], F32, name=f"ags_in{q}")
                      for q in range(4)]
            ags_out = [pgd.tile([NCORES, 128, 4], F32, addr_space="Shared",
                                name=f"ags_out{q}") for q in range(4)]

            pab_ctx = tc.tile_pool(name="pab", bufs=1)
            pab = pab_ctx.__enter__()
            hbf0 = pab.tile([128, KC, BPC], BF)              # h0 in bf16
            bihT = pab.tile([128, MC3], F32)
            bhhB = pab.tile([128, MC3, BPC], F32)
            idx_sb = pab.tile([128, 2], I32)
            xg_sb = pab.tile([128, MC3, RPC], F32)           # xg' = x@W_ih.T + b_ih
            whh_sb = pab.tile([128, KC, 3 * H], BF)
            nc.sync.dma_start(whh_sb[:], whh_t.rearrange("(a p) m -> p a m", p=128))

            nc.sync.dma_start(bihT[:], b_ihT[:])
            nc.sync.dma_start(bhhB[:], b_hhB[:])
            nc.sync.dma_start(idx_sb[:], idx.rearrange("(j p) -> p j", p=128))
            nc.vector.tensor_copy(hbf0[:], hT[:])

            # ================= Phase A: gather + relu + transpose + xg =====
            with nc.named_scope("ph_a"), \
                 tc.tile_pool(name="pa", bufs=1) as pa, \
                 tc.tile_pool(name="pa_ps", bufs=2, space="PSUM") as paps, \
                 tc.tile_pool(name="pa_ps2", bufs=2, space="PSUM") as paps2:
                x_sb = pa.tile([128, 2, H], F32)
                ident = pa.tile([128, 128], F32)
                make_identity(nc, ident[:])
                for j in range(2):
                    nc.gpsimd.indirect_dma_start(
                        out=x_sb[:, j, :], out_offset=None,
                        in_=emb[:],
                        in_offset=bass.IndirectOffsetOnAxis(ap=idx_sb[:, j:j + 1], axis=0),
                    )
                wih_sb = pa.tile([128, KC, 3 * H], BF)
                nc.sync.dma_start(wih_sb[:], wih_t.rearrange("(a p) m -> p a m", p=128))

                # xT[p, kc, r] = relu(x[r, 128*kc+p]) for r = 4t + bl
                xT = pa.tile([128, KC, RPC], BF)
                for j in range(2):
                    for k in range(KC):
                        tp = paps.tile([128, 128], F32, tag="tp")
                        nc.tensor.transpose(tp[:], x_sb[:, j, 128 * k:128 * k + 128],
                                            ident[:])
                        nc.scalar.activation(xT[:, k, 128 * j:128 * j + 128], tp[:],
                                             mybir.ActivationFunctionType.Relu)

                # xg'[p, m, r] = sum_k W_ih[128m+p, k] x[r, k] + b_ih[128m+p]
                for m in range(MC3):
                    pxg = paps2.tile([128, RPC], F32, tag="pxg")
                    for k in range(KC):
                        nc.tensor.matmul(pxg[:], wih_sb[:, k, 128 * m:128 * m + 128],
                                         xT[:, k, :],
                                         start=(k == 0), stop=(k == KC - 1))
                    nc.vector.tensor_add(
                        out=xg_sb[:, m, :], in0=pxg[:],
                        in1=bihT[:, m:m + 1].to_broadcast([128, RPC]))

            # ================= Phase B: GRU recurrence =====================
            with nc.named_scope("rec"), \
                 tc.tile_pool(name="pb", bufs=3) as pb, \
                 tc.tile_pool(name="pb_ps", bufs=2, space="PSUM") as pbps:
                for t in range(T):
                    rhs = hbf0 if t == 0 else slab[:, :, t - 1, :]
                    hg = pbps.tile([128, MC3, BPC], F32, tag="hg")
                    for m in range(MC3):
                        for k in range(KC):
                            nc.tensor.matmul(
                                hg[:, m, :], whh_sb[:, k, 128 * m:128 * m + 128],
                                (rhs[:, k, :] if t == 0 else slab[:, k, t - 1, :]),
                                start=(k == 0), stop=(k == KC - 1))
                    # gates (fp32)
                    hgb = pb.tile([128, MC3, BPC], F32, tag="hgb")
                    nc.vector.tensor_add(out=hgb[:], in0=hg[:], in1=bhhB[:])
                    rzs = pb.tile([128, 16, BPC], F32, tag="rzs")
                    nc.vector.tensor_add(out=rzs[:], in0=hgb[:, 0:16, :],
                                         in1=xg_sb[:, 0:16, 4 * t:4 * t + BPC])
                    rz = pb.tile([128, 16, BPC], F32, tag="rz")
                    nc.scalar.activation(rz[:], rzs[:],
                                         mybir.ActivationFunctionType.Sigmoid)
                    ns = pb.tile([128, KC, BPC], F32, tag="ns")
                    nc.vector.tensor_mul(out=ns[:], in0=rz[:, 0:KC, :],
                                         in1=hgb[:, 16:24, :])
                    nc.vector.tensor_add(out=ns[:], in0=ns[:],
                                         in1=xg_sb[:, 16:24, 4 * t:4 * t + BPC])
                    n_t = pb.tile([128, KC, BPC], F32, tag="n_t")
                    nc.scalar.activation(n_t[:], ns[:],
                                         mybir.ActivationFunctionType.Tanh)
                    d = pb.tile([128, KC, BPC], F32, tag="d")
                    nc.vector.tensor_tensor(out=d[:], in0=hT[:], in1=n_t[:],
                                            op=mybir.AluOpType.subtract)
                    nc.vector.tensor_mul(out=d[:], in0=d[:], in1=rz[:, 8:16, :])
                    nc.vector.tensor_add(out=hT[:], in0=n_t[:], in1=d[:])
                    nc.vector.tensor_copy(slab[:, :, t, :], hT[:])

                    if t == 31:
                        nc.sync.dma_start(agin1[:], slab[:, :, 0:32, :])
                        nc.gpsimd.collective_compute(
                            "AllGather", mybir.AluOpType.bypass,
                            replica_groups=[list(range(NCORES))],
                            ins=[agin1.opt()], outs=[agout1.opt()])
                        for c in range(NCORES):
                            nc.sync.dma_start(hsall[:, :, c, 0:32, :], agout1[c])
                nc.sync.dma_start(out_h.rearrange("(a p) b -> p a b", p=128), hT[:])
                nc.sync.dma_start(agin2[:], slab[:, :, 32:64, :])
                nc.gpsimd.collective_compute(
                    "AllGather", mybir.AluOpType.bypass,
                    replica_groups=[list(range(NCORES))],
                    ins=[agin2.opt()], outs=[agout2.opt()])
                for c in range(NCORES):
                    nc.sync.dma_start(hsall[:, :, c, 32:64, :], agout2[c])

            pab_ctx.__exit__(None, None, None)

            # ================= Phase C: projection + log_softmax ===========
            with nc.named_scope("proj"), \
                 tc.tile_pool(name="pc", bufs=1) as pc, \
                 tc.tile_pool(name="pc_sl", bufs=8) as pcs, \
                 tc.tile_pool(name="pc_sm", bufs=4) as pcm, \
                 tc.tile_pool(name="pc_ps", bufs=2, space="PSUM") as pcps:
                oww_sb = pc.tile([128, KC, VPC], BF)
                for k in range(KC):
                    nc.sync.dma_start(
                        oww_sb[:, k, :],
                        oww.rearrange("(a p) v -> p a v", p=128)[:, k, :])
                obB = pc.tile([128, VPC], F32)
                nc.sync.dma_start(obB[:], ob[None, :].to_broadcast([128, VPC]))

                q_tiles = []  # (block, slab) of current quarter
                order = [(c2, 0) for c2 in range(NCORES)] + \
                        [(c2, 1) for c2 in range(NCORES)]
                for mi, (c2, half) in enumerate(order):
                    mq = mi % 4
                    if mq == 0:
                        s_q = pcm.tile([128, 4], F32, tag="s_q")
                        q_tiles = []
                    lg = pcs.tile([128, VPC], BF, tag="lg")
                    q_tiles.append((2 * c2 + half, lg))
                    spart = pcm.tile([128, NV], F32, tag="spart")
                    for g in range(NV // 4):
                        pp = pcps.tile([128, 4, 512], F32, tag="pp")
                        for k in range(KC):
                            for j in range(4):
                                n = 4 * g + j
                                nn = min(512, VPC - 512 * n)
                                nc.tensor.matmul(
                                    pp[:, j, :nn],
                                    hsall[:, k, c2, 32 * half:32 * half + 32, :],
                                    oww_sb[:, k, 512 * n:512 * n + nn],
                                    start=(k == 0), stop=(k == KC - 1))
                        for j in range(4):
                            n = 4 * g + j
                            nn = min(512, VPC - 512 * n)
                            nc.vector.tensor_add(out=lg[:, 512 * n:512 * n + nn],
                                                 in0=pp[:, j, :nn],
                                                 in1=obB[:, 512 * n:512 * n + nn])
                            esc = pcm.tile([128, 512], F32, tag="esc")
                            nc.scalar.activation(esc[:, :nn],
                                                 lg[:, 512 * n:512 * n + nn],
                                                 mybir.ActivationFunctionType.Exp,
                                                 accum_out=spart[:, n:n + 1])
                    nc.vector.reduce_sum(s_q[:, mq:mq + 1], spart[:],
                                         axis=mybir.AxisListType.X)

                    if mq == 3:
                        qq = mi // 4
                        nc.sync.dma_start(ags_in[qq][:], s_q[:])
                        nc.gpsimd.collective_compute(
                            "AllGather", mybir.AluOpType.bypass,
                            replica_groups=[list(range(NCORES))],
                            ins=[ags_in[qq].opt()], outs=[ags_out[qq].opt()])
                        sall = pcm.tile([128, 4, NCORES], F32, tag="sall")
                        nc.sync.dma_start(
                            sall[:], ags_out[qq][:].rearrange("c p m -> p m c"))
                        s_g = pcm.tile([128, 4], F32, tag="s_g")
                        nc.vector.reduce_sum(s_g[:], sall[:],
                                             axis=mybir.AxisListType.X)
                        nb = pcm.tile([128, 4], F32, tag="nb")
                        nc.scalar.activation(nb[:], s_g[:],
                                             mybir.ActivationFunctionType.Ln)
                        nc.vector.tensor_scalar_mul(nb[:], nb[:], -1.0)
                        # normalize + write out the 4 row-tiles of this quarter
                        for i, (blk, lgq) in enumerate(q_tiles):
                            for n in range(NV):
                                nn = min(512, VPC - 512 * n)
                                ofl = pcm.tile([128, 512], F32, tag="ofl")
                                nc.scalar.activation(
                                    ofl[:, :nn], lgq[:, 512 * n:512 * n + nn],
                                    mybir.ActivationFunctionType.Identity,
                                    bias=nb[:, i:i + 1])
                                nc.sync.dma_start(
                                    out_lp[128 * blk:128 * blk + 128,
                                           512 * n:512 * n + nn],
                                    ofl[:, :nn])
    nc.compile()
    return nc


def _get_nc():
    if "nc" not in _CACHE:
        _CACHE["nc"] = _build()
    return _CACHE["nc"]


def kernel(encoder_outputs, encoder_hidden, target_tensor, embedding,
           W_ih, W_hh, b_ih, b_hh, out_W, out_b):
    del encoder_outputs
    bf = ml_dtypes.bfloat16
    emb = np.ascontiguousarray(np.asarray(embedding, dtype=np.float32))
    tt = np.asarray(target_tensor)
    eh = np.asarray(encoder_hidden, dtype=np.float32)
    wih_t = np.ascontiguousarray(np.asarray(W_ih, np.float32).T).astype(bf)
    whh_t = np.ascontiguousarray(np.asarray(W_hh, np.float32).T).astype(bf)
    b_ih = np.asarray(b_ih, np.float32)
    b_hh = np.asarray(b_hh, np.float32)
    out_W = np.asarray(out_W, np.float32)
    out_b = np.asarray(out_b, np.float32)

    b_ihT = np.ascontiguousarray(b_ih.reshape(MC3, 128).T)
    b_hhB = np.ascontiguousarray(
        np.broadcast_to(b_hh.reshape(MC3, 128).T[:, :, None], (128, MC3, BPC)))

    # teacher-forcing inputs: SOS then target[:, :-1]
    ins = np.concatenate(
        [np.full((B, 1), SOS, dtype=np.int64), tt[:, :-1].astype(np.int64)], axis=1)

    in_maps = []
    for c in range(NCORES):
        sl = slice(BPC * c, BPC * (c + 1))
        vsl = slice(VPC * c, VPC * (c + 1))
        idx_c = np.ascontiguousarray(ins[sl].T.reshape(-1)).astype(np.int32)
        h0_t = np.ascontiguousarray(eh[0, sl].T)
        oww = np.ascontiguousarray(out_W[vsl].T).astype(bf)
        in_maps.append({
            "emb": emb, "idx": idx_c, "wih_t": wih_t, "whh_t": whh_t,
            "b_ihT": b_ihT, "b_hhB": b_hhB, "h0_t": h0_t,
            "oww": oww, "ob": np.ascontiguousarray(out_b[vsl]),
        })

    nc = _get_nc()
    res = bass_utils.run_bass_kernel_spmd(
        nc, in_maps, core_ids=list(range(NCORES)),
        trace=bool(_CACHE.get("trace")))
    _CACHE["last_results"] = res

    lp_parts = []
    for c in range(NCORES):
        r = res.results[c]["out_lp"]
        # device rows: [c2, half, t32, b, V] -> [b_global, t, V]
        r = r.reshape(NCORES, 2, 32, BPC, VPC).transpose(0, 3, 1, 2, 4)
        lp_parts.append(r.reshape(B, T, VPC))
    log_probs = np.concatenate(lp_parts, axis=-1)
    hidden = np.concatenate(
        [res.results[c]["out_h"].T for c in range(NCORES)], axis=0)[None]
    return log_probs, hidden


# revision 9
# speedup vs baseline: 1.0247x; 1.0140x over previous
"""DecoderRNN (teacher-forced GRU decoder + vocab projection + log_softmax)
on 8 Trainium2 NeuronCores.

Sharding: batch-split GRU recurrence (4 examples per core), vocab-split
output projection (4000 vocab per core). Hidden states are AllGathered in
two time-chunks; log-softmax normalization uses one small AllGather of
per-row sum(exp(logit)) per row-quarter (logits are O(1) so no max shift
is needed for fp32 exp).

Self-contained: hardcodes all shapes; takes full inputs, returns full
outputs (log_probs [32,64,32000] f32, hidden [1,32,1024] f32).
"""
import numpy as np
import ml_dtypes

import concourse.bass as bass
import concourse.bacc as bacc
import concourse.tile as tile
from concourse import mybir
from concourse import bass_utils
from concourse.masks import make_identity

BF = mybir.dt.bfloat16
F32 = mybir.dt.float32
I32 = mybir.dt.int32

NCORES = 8
B, T, H, V = 32, 64, 1024, 32000
BPC = B // NCORES          # 4 examples per core
VPC = V // NCORES          # 4000 vocab per core
ROWS = B * T               # 2048
RPC = BPC * T              # 256 rows gathered per core
KC = H // 128              # 8 k-chunks
MC3 = 3 * H // 128         # 24 m-chunks of gates
NT = 16                    # projection row-tiles of 128
NV = (VPC + 511) // 512    # 8 vocab chunks (last = 416)
SOS = 1

_CACHE = {}


def _build():
    nc = bacc.Bacc("TRN2", target_bir_lowering=False, debug=False,
                   num_devices=NCORES)
    emb = nc.dram_tensor("emb", [V, H], F32, kind="ExternalInput").ap()
    idx = nc.dram_tensor("idx", [RPC], I32, kind="ExternalInput").ap()
    wih_t = nc.dram_tensor("wih_t", [H, 3 * H], BF, kind="ExternalInput").ap()
    whh_t = nc.dram_tensor("whh_t", [H, 3 * H], BF, kind="ExternalInput").ap()
    b_ihT = nc.dram_tensor("b_ihT", [128, MC3], F32, kind="ExternalInput").ap()
    b_hhB = nc.dram_tensor("b_hhB", [128, MC3, BPC], F32, kind="ExternalInput").ap()
    h0_t = nc.dram_tensor("h0_t", [H, BPC], F32, kind="ExternalInput").ap()
    oww = nc.dram_tensor("oww", [H, VPC], BF, kind="ExternalInput").ap()
    ob = nc.dram_tensor("ob", [VPC], F32, kind="ExternalInput").ap()
    out_lp = nc.dram_tensor("out_lp", [ROWS, VPC], F32, kind="ExternalOutput").ap()
    out_h = nc.dram_tensor("out_h", [H, BPC], F32, kind="ExternalOutput").ap()

    with tile.TileContext(nc) as tc:
        with tc.tile_pool(name="pg", bufs=1) as pg, \
             tc.tile_pool(name="pg_dram", bufs=1, space="DRAM") as pgd:
            # ---- long-lived tiles ----
            hsall = pg.tile([128, KC, NCORES, T, BPC], BF)   # gathered hidden states
            slab = pg.tile([128, KC, T, BPC], BF)            # own hidden states
            hT = pg.tile([128, KC, BPC], F32)                # current h (fp32)

            nc.sync.dma_start(hT[:], h0_t.rearrange("(a p) b -> p a b", p=128))

            # DRAM bounce buffers for collectives
            agin1 = pgd.tile([128, KC, 32, BPC], BF)
            agin2 = pgd.tile([128, KC, 32, BPC], BF)
            agout1 = pgd.tile([NCORES, 128, KC, 32, BPC], BF, addr_space="Shared")
            agout2 = pgd.tile([NCORES, 128, KC, 32, BPC], BF, addr_space="Shared")
            ags_in = [pgd.tile([128, 2], F32, name=f"ags_in{q}")
                      for q in range(8)]
            ags_out = [pgd.tile([NCORES, 128, 2], F32, addr_space="Shared",
                                name=f"ags_out{q}") for q in range(8)]

            pab_ctx = tc.tile_pool(name="pab", bufs=1)
            pab = pab_ctx.__enter__()
            hbf0 = pab.tile([128, KC, BPC], BF)              # h0 in bf16
            bihT = pab.tile([128, MC3], F32)
            bhhB = pab.tile([128, MC3, BPC], F32)
            idx_sb = pab.tile([128, 2], I32)
            xg_sb = pab.tile([128, MC3, RPC], F32)           # xg' = x@W_ih.T + b_ih
            whh_sb = pab.tile([128, KC, 3 * H], BF)
            nc.sync.dma_start(whh_sb[:], whh_t.rearrange("(a p) m -> p a m", p=128))

            nc.sync.dma_start(bihT[:], b_ihT[:])
            nc.sync.dma_start(bhhB[:], b_hhB[:])
            nc.sync.dma_start(idx_sb[:], idx.rearrange("(j p) -> p j", p=128))
            nc.vector.tensor_copy(hbf0[:], hT[:])

            # ================= Phase A: gather + relu + transpose + xg =====
            with nc.named_scope("ph_a"), \
                 tc.tile_pool(name="pa", bufs=1) as pa, \
                 tc.tile_pool(name="pa_ps", bufs=2, space="PSUM") as paps, \
                 tc.tile_pool(name="pa_ps2", bufs=2, space="PSUM") as paps2:
                x_sb = pa.tile([128, 2, H], F32)
                ident = pa.tile([128, 128], F32)
                make_identity(nc, ident[:])
                for j in range(2):
                    nc.gpsimd.indirect_dma_start(
                        out=x_sb[:, j, :], out_offset=None,
                        in_=emb[:],
                        in_offset=bass.IndirectOffsetOnAxis(ap=idx_sb[:, j:j + 1], axis=0),
                    )
                wih_sb = pa.tile([128, KC, 3 * H], BF)
                nc.sync.dma_start(wih_sb[:], wih_t.rearrange("(a p) m -> p a m", p=128))

                # xT[p, kc, r] = relu(x[r, 128*kc+p]) for r = 4t + bl
                xT = pa.tile([128, KC, RPC], BF)
                for j in range(2):
                    for k in range(KC):
                        tp = paps.tile([128, 128], F32, tag="tp")
                        nc.tensor.transpose(tp[:], x_sb[:, j, 128 * k:128 * k + 128],
                                            ident[:])
                        nc.scalar.activation(xT[:, k, 128 * j:128 * j + 128], tp[:],
                                             mybir.ActivationFunctionType.Relu)

                # xg'[p, m, r] = sum_k W_ih[128m+p, k] x[r, k] + b_ih[128m+p]
                for m in range(MC3):
                    pxg = paps2.tile([128, RPC], F32, tag="pxg")
                    for k in range(KC):
                        nc.tensor.matmul(pxg[:], wih_sb[:, k, 128 * m:128 * m + 128],
                                         xT[:, k, :],
                                         start=(k == 0), stop=(k == KC - 1))
                    nc.vector.tensor_add(
                        out=xg_sb[:, m, :], in0=pxg[:],
                        in1=bihT[:, m:m + 1].to_broadcast([128, RPC]))

            # ================= Phase B: GRU recurrence =====================
            with nc.named_scope("rec"), \
                 tc.tile_pool(name="pb", bufs=3) as pb, \
                 tc.tile_pool(name="pb_ps", bufs=2, space="PSUM") as pbps:
                for t in range(T):
                    rhs = hbf0 if t == 0 else slab[:, :, t - 1, :]
                    hg = pbps.tile([128, MC3, BPC], F32, tag="hg")
                    for m in range(MC3):
                        for k in range(KC):
                            nc.tensor.matmul(
                                hg[:, m, :], whh_sb[:, k, 128 * m:128 * m + 128],
                                (rhs[:, k, :] if t == 0 else slab[:, k, t - 1, :]),
                                start=(k == 0), stop=(k == KC - 1))
                    # gates (fp32)
                    hgb = pb.tile([128, MC3, BPC], F32, tag="hgb")
                    nc.vector.tensor_add(out=hgb[:], in0=hg[:], in1=bhhB[:])
                    rzs = pb.tile([128, 16, BPC], F32, tag="rzs")
                    nc.vector.tensor_add(out=rzs[:], in0=hgb[:, 0:16, :],
                                         in1=xg_sb[:, 0:16, 4 * t:4 * t + BPC])
                    rz = pb.tile([128, 16, BPC], F32, tag="rz")
                    nc.scalar.activation(rz[:], rzs[:],
                                         mybir.ActivationFunctionType.Sigmoid)
                    ns = pb.tile([128, KC, BPC], F32, tag="ns")
                    nc.vector.tensor_mul(out=ns[:], in0=rz[:, 0:KC, :],
                                         in1=hgb[:, 16:24, :])
                    nc.vector.tensor_add(out=ns[:], in0=ns[:],
                                         in1=xg_sb[:, 16:24, 4 * t:4 * t + BPC])
                    n_t = pb.tile([128, KC, BPC], F32, tag="n_t")
                    nc.scalar.activation(n_t[:], ns[:],
                                         mybir.ActivationFunctionType.Tanh)
                    d = pb.tile([128, KC, BPC], F32, tag="d")
                    nc.vector.tensor_tensor(out=d[:], in0=hT[:], in1=n_t[:],
                                            op=mybir.AluOpType.subtract)
                    nc.vector.tensor_mul(out=d[:], in0=d[:], in1=rz[:, 8:16, :])
                    nc.vector.tensor_add(out=hT[:], in0=n_t[:], in1=d[:])
                    nc.vector.tensor_copy(slab[:, :, t, :], hT[:])

                    if t == 31:
                        nc.sync.dma_start(agin1[:], slab[:, :, 0:32, :])
                        nc.gpsimd.collective_compute(
                            "AllGather", mybir.AluOpType.bypass,
                            replica_groups=[list(range(NCORES))],
                            ins=[agin1.opt()], outs=[agout1.opt()])
                        for c in range(NCORES):
                            nc.sync.dma_start(hsall[:, :, c, 0:32, :], agout1[c])
                nc.sync.dma_start(out_h.rearrange("(a p) b -> p a b", p=128), hT[:])
                nc.sync.dma_start(agin2[:], slab[:, :, 32:64, :])
                nc.gpsimd.collective_compute(
                    "AllGather", mybir.AluOpType.bypass,
                    replica_groups=[list(range(NCORES))],
                    ins=[agin2.opt()], outs=[agout2.opt()])
                for c in range(NCORES):
                    nc.sync.dma_start(hsall[:, :, c, 32:64, :], agout2[c])

            pab_ctx.__exit__(None, None, None)

            # ================= Phase C: projection + log_softmax ===========
            with nc.named_scope("proj"), \
                 tc.tile_pool(name="pc", bufs=1) as pc, \
                 tc.tile_pool(name="pc_sl", bufs=8) as pcs, \
                 tc.tile_pool(name="pc_sm", bufs=4) as pcm, \
                 tc.tile_pool(name="pc_ps", bufs=2, space="PSUM") as pcps:
                oww_sb = pc.tile([128, KC, VPC], BF)
                for k in range(KC):
                    nc.sync.dma_start(
                        oww_sb[:, k, :],
                        oww.rearrange("(a p) v -> p a v", p=128)[:, k, :])
                obB = pc.tile([128, VPC], F32)
                nc.sync.dma_start(obB[:], ob[None, :].to_broadcast([128, VPC]))

                q_tiles = []  # (block, slab) of current quarter
                order = [(c2, 0) for c2 in range(NCORES)] + \
                        [(c2, 1) for c2 in range(NCORES)]
                for mi, (c2, half) in enumerate(order):
                    mq = mi % 2
                    if mq == 0:
                        s_q = pcm.tile([128, 2], F32, tag="s_q")
                        q_tiles = []
                    lg = pcs.tile([128, VPC], BF, tag="lg")
                    q_tiles.append((2 * c2 + half, lg))
                    spart = pcm.tile([128, NV], F32, tag="spart")
                    for g in range(NV // 4):
                        pp = pcps.tile([128, 4, 512], F32, tag="pp")
                        for k in range(KC):
                            for j in range(4):
                                n = 4 * g + j
                                nn = min(512, VPC - 512 * n)
                                nc.tensor.matmul(
                                    pp[:, j, :nn],
                                    hsall[:, k, c2, 32 * half:32 * half + 32, :],
                                    oww_sb[:, k, 512 * n:512 * n + nn],
                                    start=(k == 0), stop=(k == KC - 1))
                        for j in range(4):
                            n = 4 * g + j
                            nn = min(512, VPC - 512 * n)
                            nc.vector.tensor_add(out=lg[:, 512 * n:512 * n + nn],
                                                 in0=pp[:, j, :nn],
                                                 in1=obB[:, 512 * n:512 * n + nn])
                            esc = pcm.tile([128, 512], F32, tag="esc")
                            nc.scalar.activation(esc[:, :nn],
                                                 lg[:, 512 * n:512 * n + nn],
                                                 mybir.ActivationFunctionType.Exp,
                                                 accum_out=spart[:, n:n + 1])
                    nc.vector.reduce_sum(s_q[:, mq:mq + 1], spart[:],
                                         axis=mybir.AxisListType.X)

                    if mq == 1:
                        qq = mi // 2
                        nc.sync.dma_start(ags_in[qq][:], s_q[:])
                        nc.gpsimd.collective_compute(
                            "AllGather", mybir.AluOpType.bypass,
                            replica_groups=[list(range(NCORES))],
                            ins=[ags_in[qq].opt()], outs=[ags_out[qq].opt()])
                        sall = pcm.tile([128, 2, NCORES], F32, tag="sall")
                        nc.sync.dma_start(
                            sall[:], ags_out[qq][:].rearrange("c p m -> p m c"))
                        s_g = pcm.tile([128, 2], F32, tag="s_g")
                        nc.vector.reduce_sum(s_g[:], sall[:],
                                             axis=mybir.AxisListType.X)
                        nb = pcm.tile([128, 2], F32, tag="nb")
                        nc.scalar.activation(nb[:], s_g[:],
                                             mybir.ActivationFunctionType.Ln)
                        nc.vector.tensor_scalar_mul(nb[:], nb[:], -1.0)
                        # normalize + write out the 4 row-tiles of this quarter
                        for i, (blk, lgq) in enumerate(q_tiles):
                            for n in range(NV):
                                nn = min(512, VPC - 512 * n)
                                ofl = pcm.tile([128, 512], F32, tag="ofl")
                                nc.scalar.activation(
                                    ofl[:, :nn], lgq[:, 512 * n:512 * n + nn],
                                    mybir.ActivationFunctionType.Identity,
                                    bias=nb[:, i:i + 1])
                                nc.sync.dma_start(
                                    out_lp[128 * blk:128 * blk + 128,
                                           512 * n:512 * n + nn],
                                    ofl[:, :nn])
    nc.compile()
    return nc


def _get_nc():
    if "nc" not in _CACHE:
        _CACHE["nc"] = _build()
    return _CACHE["nc"]


def kernel(encoder_outputs, encoder_hidden, target_tensor, embedding,
           W_ih, W_hh, b_ih, b_hh, out_W, out_b):
    del encoder_outputs
    bf = ml_dtypes.bfloat16
    emb = np.ascontiguousarray(np.asarray(embedding, dtype=np.float32))
    tt = np.asarray(target_tensor)
    eh = np.asarray(encoder_hidden, dtype=np.float32)
    wih_t = np.ascontiguousarray(np.asarray(W_ih, np.float32).T).astype(bf)
    whh_t = np.ascontiguousarray(np.asarray(W_hh, np.float32).T).astype(bf)
    b_ih = np.asarray(b_ih, np.float32)
    b_hh = np.asarray(b_hh, np.float32)
    out_W = np.asarray(out_W, np.float32)
    out_b = np.asarray(out_b, np.float32)

    b_ihT = np.ascontiguousarray(b_ih.reshape(MC3, 128).T)
    b_hhB = np.ascontiguousarray(
        np.broadcast_to(b_hh.reshape(MC3, 128).T[:, :, None], (128, MC3, BPC)))

    # teacher-forcing inputs: SOS then target[:, :-1]
    ins = np.concatenate(
        [np.full((B, 1), SOS, dtype=np.int64), tt[:, :-1].astype(np.int64)], axis=1)

    in_maps = []
    for c in range(NCORES):
        sl = slice(BPC * c, BPC * (c + 1))
        vsl = slice(VPC * c, VPC * (c + 1))
        idx_c = np.ascontiguousarray(ins[sl].T.reshape(-1)).astype(np.int32)
        h0_t = np.ascontiguousarray(eh[0, sl].T)
        oww = np.ascontiguousarray(out_W[vsl].T).astype(bf)
        in_maps.append({
            "emb": emb, "idx": idx_c, "wih_t": wih_t, "whh_t": whh_t,
            "b_ihT": b_ihT, "b_hhB": b_hhB, "h0_t": h0_t,
            "oww": oww, "ob": np.ascontiguousarray(out_b[vsl]),
        })

    nc = _get_nc()
    res = bass_utils.run_bass_kernel_spmd(
        nc, in_maps, core_ids=list(range(NCORES)),
        trace=bool(_CACHE.get("trace")))
    _CACHE["last_results"] = res

    lp_parts = []
    for c in range(NCORES):
        r = res.results[c]["out_lp"]
        # device rows: [c2, half, t32, b, V] -> [b_global, t, V]
        r = r.reshape(NCORES, 2, 32, BPC, VPC).transpose(0, 3, 1, 2, 4)
        lp_parts.append(r.reshape(B, T, VPC))
    log_probs = np.concatenate(lp_parts, axis=-1)
    hidden = np.concatenate(
        [res.results[c]["out_h"].T for c in range(NCORES)], axis=0)[None]
    return log_probs, hidden
